# revision 27
# baseline (speedup 1.0000x reference)
"""Attention with host-folded QK^T kernel + pair-wise V dedup AllGather.

v12: the K projection never runs on device. scores = q·k^T with
q = x W_q, k = x W_k factors as x (W_q W_k^T) x^T, so the host
precomputes M = 64·W_q W_k^T (fp32 matmul, then fp16 — the 64×
scale keeps M's ~1e-5-magnitude entries out of fp16 subnormals;
the exp activation scale absorbs the 1/64). Each core computes
Q' = x_q M for its query half (same cost as the old Q projection)
and scores come from Q'·x_k^T against the xh slabs directly —
the entire 32µs redundant full-K projection is gone.

Keys are RANK-RELATIVE: the host builds each core's xh as
[own 1024 tokens | peer 1024 tokens], which (a) makes xq redundant
(A1/A2 read the first two xh slabs), and (b) lets V' for the own
half live entirely on-core (vpA) — only the PEER half of V' comes
back from the 2-rank AllGather. The gather output is rank-ordered,
so the peer slot index depends on the rank; two complementary
cond= predicated DMAs (skipped DMAs still increment their
semaphore) funnel the right slot into vpB with fully static APs.
Attention is an order-free reduction over keys, so rank-relative
key order changes nothing downstream.

Schedule notes:
- all inputs arrive host-pre-arranged in SBUF slab layout, one
  large-line DMA each; gpsimd (the slow ~78GB/s queue) carries
  only 2 of the 6 wv chunks.
- ps_sc is allocated OUTSIDE the psa scope and the scores loop
  runs INSIDE it, so no pool-close fence sits between A2 and
  scores (measured 1.05us of PE idle otherwise); the psa close
  lands at the scores->out boundary instead.
- PSUM budget: wps(1) + psa(5) + ps_sc(2) = 8 banks in phase A;
  ps_out(3x2) reuses psa's banks in the out phase.
- vpA and vpB are separate tiles: a single tile written by two
  different DMA queues loses one of the matmul waits (HW 1-wait
  limit) — observed as a cold-run race.
- out phase: denominator run (cols 512:770) first, so the recip
  and the cols-512:768 normalize overlap the cols-0:512 run.
"""

import numpy as np

import concourse.bass as bass
import concourse.mybir as mybir
import concourse.tile as tile
from concourse import bacc
from concourse.bass_utils import run_bass_kernel_spmd

N_CORES = 8
B, N, D, OUT = 4, 2048, 768, 768
NQ = N // 2
P = 128
DC = D // P
KC = N // P
HKC = KC // 2  # k-chunks per half
F32 = mybir.dt.float32
FP16 = mybir.dt.float16
PAIRS = [[0, 1], [2, 3], [4, 5], [6, 7]]

M_SCALE = 64.0  # host folds this into M; exp scale divides it back out


def build_attention_nc():
    nc = bacc.Bacc("TRN2", target_bir_lowering=False, debug=False)
    # Inputs host-pre-arranged in SBUF slab layout [p, dc, n]; xh is
    # rank-relative: slabs 0-1 = own 1024 tokens, 2-3 = peer tokens.
    xh = nc.dram_tensor("xh", [4, P, DC * 512], FP16, kind="ExternalInput")
    mw = nc.dram_tensor("mw", [P, DC * D], FP16, kind="ExternalInput")
    wvi = nc.dram_tensor("wvi", [P, 2 * DC * 384], FP16, kind="ExternalInput")
    out = nc.dram_tensor("out", [NQ, OUT], F32, kind="ExternalOutput")

    with tile.TileContext(nc) as tc:
        with (
            tc.tile_pool(name="persist", bufs=1) as persist,
            tc.tile_pool(name="slabs", bufs=4) as slabs,
            tc.tile_pool(name="wpool", bufs=1) as wpool,
            tc.tile_pool(name="expp", bufs=34) as expp,
            tc.tile_pool(name="obp", bufs=3) as obp,
            tc.tile_pool(name="smallp", bufs=4) as smallp,
            tc.tile_pool(name="ps_sc", bufs=2, space="PSUM") as ps_sc,
            tc.tile_pool(name="dpool", bufs=1, space="DRAM") as dpool,
        ):
            # Q'^T[d,q], one tile per 512-query half so the scores phase
            # never waits on the other half's psum drain
            qpt = [
                persist.tile([P, DC, 512], FP16, name=f"qpt{s}")
                for s in range(2)
            ]
            # V' in rank-relative key order: vpA = own half (local only),
            # vpB = peer half (from the gather)
            vpA = persist.tile([P, HKC, OUT + 2], FP16, name="vpA")
            vpB = persist.tile([P, HKC, OUT + 2], FP16, name="vpB")

            vpb_in = dpool.tile([P, HKC, OUT + 2], FP16)
            vpb_out = dpool.tile([2, P, HKC, OUT + 2], FP16)

            wv_sb = wpool.tile([P, 2, DC, 384], FP16)
            mw_sb = wpool.tile([P, DC, D], FP16)

            # HAM warmup; memset on gpsimd, which boots earliest
            warm = wpool.tile([P, 512], FP16, name="warm")
            nc.gpsimd.memset(warm, 1.0)

            ones_sc = persist.tile([P, 1], F32, name="ones_sc")
            nc.vector.memset(ones_sc, 1.0)
            zero_sc = persist.tile([P, 1], F32, name="zero_sc")
            nc.vector.memset(zero_sc, 0.0)

            ets = {}
            with tc.tile_pool(name="psa", bufs=5, space="PSUM") as psa:
                wps = psa.tile([P, 512], F32, name="wps", bufs=1)
                for i in range(11):
                    nc.tensor.matmul(
                        wps, warm[:, 0:P], warm, start=(i == 0), stop=(i == 10)
                    )

                # A1-critical: xh slab0 (sync) + wv (scalar 4/6, gpsimd
                # 2/6 — gpsimd is the slow ~78GB/s queue); then slab1,
                # mw, and the peer xh slabs.
                kslab_tiles = [
                    slabs.tile([P, 4, DC, P], FP16, tag="slab", name=f"kslab{s}")
                    for s in range(4)
                ]
                nc.sync.dma_start(
                    out=kslab_tiles[0][:, 0:2], in_=xh[0][:, 0 : 2 * DC * P]
                )
                nc.scalar.dma_start(
                    out=wv_sb[:, 0, 0:4, :], in_=wvi[:, 0 : 4 * 384]
                )
                nc.gpsimd.dma_start(
                    out=kslab_tiles[0][:, 2:4], in_=xh[0][:, 2 * DC * P :]
                )
                nc.sync.dma_start(
                    out=wv_sb[:, 0, 4:DC, :], in_=wvi[:, 4 * 384 : DC * 384]
                )
                nc.gpsimd.dma_start(
                    out=wv_sb[:, 1], in_=wvi[:, DC * 384 :]
                )
                nc.sync.dma_start(out=kslab_tiles[1], in_=xh[1])
                nc.scalar.dma_start(out=mw_sb, in_=mw[:, :])
                nc.sync.dma_start(out=kslab_tiles[3], in_=xh[3])
                nc.scalar.dma_start(out=kslab_tiles[2], in_=xh[2])

                # ---- A1: V' own half (earliest -> feeds the gather) ----
                # 512/256-wide runs; ps1/ps2 interleaved per dc so both
                # matmuls share the stationary token chunk (weight load
                # elides on the second). Results land directly in vpA.
                for kc in range(HKC):
                    slab = kslab_tiles[kc // 4]
                    ps1 = psa.tile([P, 512], F32, tag="psa")
                    for dc in range(DC):
                        nc.tensor.matmul(
                            ps1[:, 0:384],
                            slab[:, kc % 4, dc, :],
                            wv_sb[:, 0, dc, :],
                            start=(dc == 0),
                            stop=(dc == DC - 1),
                        )
                    nc.vector.tensor_copy(vpA[:, kc, 0:384], ps1[:, 0:384])
                for kc in range(HKC):
                    slab = kslab_tiles[kc // 4]
                    ps2 = psa.tile([P, 512], F32, tag="psa")
                    for dc in range(DC):
                        nc.tensor.matmul(
                            ps2[:, 0:384],
                            slab[:, kc % 4, dc, :],
                            wv_sb[:, 1, dc, :],
                            start=(dc == 0),
                            stop=(dc == DC - 1),
                        )
                    nc.vector.tensor_copy(vpA[:, kc, 384:OUT], ps2[:, 0:384])
                    nc.vector.tensor_copy(vpA[:, kc, OUT : OUT + 1], ones_sc)
                    nc.vector.tensor_copy(
                        vpA[:, kc, OUT + 1 : OUT + 2], zero_sc
                    )
                    nc.gpsimd.dma_start(
                        out=vpb_in[:, kc, :], in_=vpA[:, kc, :]
                    )
                nc.gpsimd.collective_compute(
                    "AllGather",
                    mybir.AluOpType.bypass,
                    replica_groups=PAIRS,
                    ins=[vpb_in.opt()],
                    outs=[vpb_out.opt()],
                )
                # Peer-half readback: the gather output is rank-ordered,
                # so rank r's peer sits in slot 1-r. Two complementary
                # predicated DMAs keep the APs static; the skipped DMA
                # still increments the semaphore, so downstream waits
                # count identically on both ranks. Both on the sync
                # queue (single-queue writers keep the matmul wait).
                me = nc.sync.partition_id() % 2
                nc.sync.dma_start(out=vpB[:], in_=vpb_out[0], cond=me)
                nc.sync.dma_start(
                    out=vpB[:], in_=vpb_out[1], cond=(me + 1) % 2
                )

                # ---- A2: Q'^T = (x_q M)^T own half ----
                for s in range(2):
                    slab = kslab_tiles[s]
                    for oc in range(DC):
                        # two psum tiles: interleaved start=True runs in
                        # one bank re-arm the zero region and wipe the
                        # partner run's partials
                        ps_lo = psa.tile([P, 512], F32, tag="psa")
                        ps_hi = psa.tile([P, 512], F32, tag="psa")
                        for dc in range(DC):
                            nc.tensor.matmul(
                                ps_lo[:, 0:256],
                                mw_sb[:, dc, oc * P : (oc + 1) * P],
                                slab[:, 0:2, dc, :],
                                start=(dc == 0),
                                stop=(dc == DC - 1),
                            )
                            nc.tensor.matmul(
                                ps_hi[:, 0:256],
                                mw_sb[:, dc, oc * P : (oc + 1) * P],
                                slab[:, 2:4, dc, :],
                                start=(dc == 0),
                                stop=(dc == DC - 1),
                            )
                        nc.vector.tensor_copy(
                            qpt[s][:, oc, 0:256], ps_lo[:, 0:256]
                        )
                        nc.vector.tensor_copy(
                            qpt[s][:, oc, 256:512], ps_hi[:, 0:256]
                        )

                # ---- scoresT: contracts over d, stationary = xh slab
                # chunks (rank-relative key order), moving = Q'^T. Runs
                # inside the psa scope (ps_sc has its own banks) so no
                # pool-close fence sits between A2 and scores.
                for bi in range(2):
                    for kc in range(KC):
                        kslab = kslab_tiles[kc // 4]
                        st = ps_sc.tile([P, 512], F32, tag="sc")
                        for dc in range(DC):
                            nc.tensor.matmul(
                                st,
                                kslab[:, kc % 4, dc, :],
                                qpt[bi][:, dc, :],
                                start=(dc == 0),
                                stop=(dc == DC - 1),
                            )
                        et = expp.tile(
                            [P, 512], FP16, tag="exp", name=f"et{bi}_{kc}"
                        )
                        nc.scalar.activation(
                            et,
                            st,
                            mybir.ActivationFunctionType.Exp,
                            scale=0.125 / M_SCALE,
                        )
                        ets[(bi, kc)] = et

            # ---- out phase: psa's banks freed above feed ps_out; the
            # pool-close fence overlaps the V-gather wait.
            with tc.tile_pool(name="ps_out", bufs=3, space="PSUM") as ps_out:
                # 8 q-chunks of 128, rotating 3 PSUM bufs. Denominator
                # run (cols 512:770) goes FIRST so the recip and the
                # 512:768 normalize overlap the 0:512 run; kc 0-7 read
                # vpA (local), kc 8-15 read vpB (gathered peer half).
                def vsrc(kc):
                    return vpA if kc < HKC else vpB

                for j in range(NQ // P):
                    bi, jj = j // 4, j % 4
                    ops = ps_out.tile(
                        [P, OUT + 2], F32, tag="out", name=f"outps{j}"
                    )
                    for kc in range(KC):
                        nc.tensor.matmul(
                            ops[:, 512 : OUT + 2],
                            ets[(bi, kc)][:, jj * P : (jj + 1) * P],
                            vsrc(kc)[:, kc % HKC, 512 : OUT + 2],
                            start=(kc == 0),
                            stop=(kc == KC - 1),
                        )
                    recip = smallp.tile([P, 1], F32, tag="recip")
                    nc.vector.reciprocal(recip, ops[:, OUT : OUT + 1])
                    ob = obp.tile([P, OUT], F32, tag="ob")
                    nc.vector.tensor_scalar_mul(
                        ob[:, 512:OUT], ops[:, 512:OUT], recip
                    )
                    for kc in range(KC):
                        nc.tensor.matmul(
                            ops[:, 0:512],
                            ets[(bi, kc)][:, jj * P : (jj + 1) * P],
                            vsrc(kc)[:, kc % HKC, 0:512],
                            start=(kc == 0),
                            stop=(kc == KC - 1),
                        )
                    nc.vector.tensor_scalar_mul(
                        ob[:, 0:512], ops[:, 0:512], recip
                    )
                    nc.sync.dma_start(
                        out=out[j * P : (j + 1) * P, :], in_=ob
                    )
    nc.finalize()
    return nc


_NC_CACHE = None


def _get_nc():
    global _NC_CACHE
    if _NC_CACHE is None:
        _NC_CACHE = build_attention_nc()
    return _NC_CACHE


def _xh_layout(a2d):
    """[D, 2048] -> [4, P, 4*DC*128], quarter-major slabs: the kc-th
    128-token quarter of a slab is a contiguous DMA prefix."""
    t = a2d.reshape(DC, P, 4, 4, P)  # dc p s q t
    t = t.transpose(2, 1, 3, 0, 4)  # s p q dc t
    return np.ascontiguousarray(t.reshape(4, P, 4 * DC * P))


def _wv_layout(a2d):
    """[D, 768] -> [P, 2*DC*384], column-half-major."""
    t = a2d.reshape(DC, P, 2, 384)  # dc p h c
    t = t.transpose(1, 2, 0, 3)  # p h dc c
    return np.ascontiguousarray(t.reshape(P, 2 * DC * 384))


def _mw_layout(a2d):
    """[D, D] -> [P, DC*D], dc-major."""
    t = a2d.reshape(DC, P, D).transpose(1, 0, 2)
    return np.ascontiguousarray(t.reshape(P, DC * D))


def make_in_maps(x, kernel):
    x = np.asarray(x, dtype=np.float32)
    w = np.asarray(kernel, dtype=np.float32)
    mw16 = (M_SCALE * (w[0] @ w[1].T)).astype(np.float16)
    mw = _mw_layout(mw16)
    wv = _wv_layout(w[2].astype(np.float16))
    in_maps = []
    for core in range(N_CORES):
        b, half = core // 2, core % 2
        xt16 = x[b].T.astype(np.float16)
        # rank-relative key order: own 1024 tokens first, then peer's
        own = xt16[:, half * NQ : (half + 1) * NQ]
        peer = xt16[:, (1 - half) * NQ : (2 - half) * NQ]
        xh = _xh_layout(np.concatenate([own, peer], axis=1))
        in_maps.append({"xh": xh, "mw": mw, "wvi": wv})
    return in_maps


def assemble_output(results):
    out = np.empty((B, N, OUT), dtype=np.float32)
    for core in range(N_CORES):
        b, half = core // 2, core % 2
        out[b, half * NQ : (half + 1) * NQ, :] = results[core]["out"]
    return out


def run_on_hw(x, kernel, trace=False):
    nc = _get_nc()
    res = run_bass_kernel_spmd(
        nc, make_in_maps(x, kernel), list(range(N_CORES)), trace=trace
    )
    return assemble_output(res.results), res


def kernel(x, kernel):
    out, _ = run_on_hw(x, kernel, trace=False)
    return out


# revision 28
# speedup vs baseline: 1.0205x; 1.0205x over previous
"""Attention with host-folded QK^T kernel + pair-wise V dedup AllGather.

v12: the K projection never runs on device. scores = q·k^T with
q = x W_q, k = x W_k factors as x (W_q W_k^T) x^T, so the host
precomputes M = 64·W_q W_k^T (fp32 matmul, then fp16 — the 64×
scale keeps M's ~1e-5-magnitude entries out of fp16 subnormals;
the exp activation scale absorbs the 1/64). Each core computes
Q' = x_q M for its query half (same cost as the old Q projection)
and scores come from Q'·x_k^T against the xh slabs directly —
the entire 32µs redundant full-K projection is gone.

Keys are RANK-RELATIVE: the host builds each core's xh as
[own 1024 tokens | peer 1024 tokens], which (a) makes xq redundant
(A1/A2 read the first two xh slabs), and (b) lets V' for the own
half live entirely on-core (vpA) — only the PEER half of V' comes
back from the 2-rank AllGather. The gather output is rank-ordered,
so the peer slot index depends on the rank; two complementary
cond= predicated DMAs (skipped DMAs still increment their
semaphore) funnel the right slot into vpB with fully static APs.
Attention is an order-free reduction over keys, so rank-relative
key order changes nothing downstream.

Schedule notes:
- all inputs arrive host-pre-arranged in SBUF slab layout, one
  large-line DMA each; gpsimd (the slow ~78GB/s queue) carries
  only 2 of the 6 wv chunks.
- ps_sc is allocated OUTSIDE the psa scope and the scores loop
  runs INSIDE it, so no pool-close fence sits between A2 and
  scores (measured 1.05us of PE idle otherwise); the psa close
  lands at the scores->out boundary instead.
- PSUM budget: wps(1) + psa(5) + ps_sc(2) = 8 banks in phase A;
  ps_out(3x2) reuses psa's banks in the out phase.
- vpA and vpB are separate tiles: a single tile written by two
  different DMA queues loses one of the matmul waits (HW 1-wait
  limit) — observed as a cold-run race.
- out phase: denominator run (cols 512:770) first, so the recip
  and the cols-512:768 normalize overlap the cols-0:512 run.
"""

import numpy as np

import concourse.bass as bass
import concourse.mybir as mybir
import concourse.tile as tile
from concourse import bacc
from concourse.bass_utils import run_bass_kernel_spmd

N_CORES = 8
B, N, D, OUT = 4, 2048, 768, 768
NQ = N // 2
P = 128
DC = D // P
KC = N // P
HKC = KC // 2  # k-chunks per half
F32 = mybir.dt.float32
FP16 = mybir.dt.float16
PAIRS = [[0, 1], [2, 3], [4, 5], [6, 7]]

M_SCALE = 64.0  # host folds this into M; exp scale divides it back out


def build_attention_nc():
    nc = bacc.Bacc("TRN2", target_bir_lowering=False, debug=False)
    # Inputs host-pre-arranged in SBUF slab layout [p, dc, n]; xh is
    # rank-relative: slabs 0-1 = own 1024 tokens, 2-3 = peer tokens.
    xh = nc.dram_tensor("xh", [4, P, DC * 512], FP16, kind="ExternalInput")
    mw = nc.dram_tensor("mw", [P, DC * D], FP16, kind="ExternalInput")
    wvi = nc.dram_tensor("wvi", [P, 2 * DC * 384], FP16, kind="ExternalInput")
    out = nc.dram_tensor("out", [NQ, OUT], F32, kind="ExternalOutput")

    with tile.TileContext(nc) as tc:
        with (
            tc.tile_pool(name="persist", bufs=1) as persist,
            tc.tile_pool(name="slabs", bufs=4) as slabs,
            tc.tile_pool(name="wpool", bufs=1) as wpool,
            tc.tile_pool(name="expp", bufs=34) as expp,
            tc.tile_pool(name="obp", bufs=3) as obp,
            tc.tile_pool(name="smallp", bufs=4) as smallp,
            tc.tile_pool(name="ps_sc", bufs=2, space="PSUM") as ps_sc,
            tc.tile_pool(name="dpool", bufs=1, space="DRAM") as dpool,
        ):
            # Q'^T[d,q], one tile per 512-query half so the scores phase
            # never waits on the other half's psum drain
            qpt = [
                persist.tile([P, DC, 512], FP16, name=f"qpt{s}")
                for s in range(2)
            ]
            # V' in rank-relative key order: vpA = own half (local only),
            # vpB = peer half (from the gather)
            vpA = persist.tile([P, HKC, OUT + 2], FP16, name="vpA")
            vpB = persist.tile([P, HKC, OUT + 2], FP16, name="vpB")

            vpb_in = dpool.tile([P, HKC, OUT + 2], FP16)
            vpb_out = dpool.tile([2, P, HKC, OUT + 2], FP16)

            wv_sb = wpool.tile([P, 2, DC, 384], FP16)
            mw_sb = wpool.tile([P, DC, D], FP16)

            # HAM warmup; memset on gpsimd, which boots earliest
            warm = wpool.tile([P, 512], FP16, name="warm")
            nc.gpsimd.memset(warm, 1.0)

            ones_sc = persist.tile([P, 1], F32, name="ones_sc")
            nc.vector.memset(ones_sc, 1.0)
            zero_sc = persist.tile([P, 1], F32, name="zero_sc")
            nc.vector.memset(zero_sc, 0.0)

            ets = {}
            with tc.tile_pool(name="psa", bufs=5, space="PSUM") as psa:
                wps = psa.tile([P, 512], F32, name="wps", bufs=1)
                for i in range(11):
                    nc.tensor.matmul(
                        wps, warm[:, 0:P], warm, start=(i == 0), stop=(i == 10)
                    )

                # A1-critical: xh slab0 (sync) + wv (scalar 4/6, gpsimd
                # 2/6 — gpsimd is the slow ~78GB/s queue); then slab1,
                # mw, and the peer xh slabs.
                kslab_tiles = [
                    slabs.tile([P, 4, DC, P], FP16, tag="slab", name=f"kslab{s}")
                    for s in range(4)
                ]
                nc.sync.dma_start(out=kslab_tiles[0], in_=xh[0])
                nc.scalar.dma_start(
                    out=wv_sb[:, 0], in_=wvi[:, 0 : DC * 384]
                )
                nc.gpsimd.dma_start(
                    out=wv_sb[:, 1], in_=wvi[:, DC * 384 :]
                )
                nc.sync.dma_start(out=kslab_tiles[1], in_=xh[1])
                nc.scalar.dma_start(out=mw_sb, in_=mw[:, :])
                nc.sync.dma_start(out=kslab_tiles[3], in_=xh[3])
                nc.scalar.dma_start(out=kslab_tiles[2], in_=xh[2])

                # ---- A1: V' own half (earliest -> feeds the gather) ----
                # 512/256-wide runs; ps1/ps2 interleaved per dc so both
                # matmuls share the stationary token chunk (weight load
                # elides on the second). Results land directly in vpA.
                for kc in range(HKC):
                    slab = kslab_tiles[kc // 4]
                    ps1 = psa.tile([P, 512], F32, tag="psa")
                    for dc in range(DC):
                        nc.tensor.matmul(
                            ps1[:, 0:384],
                            slab[:, kc % 4, dc, :],
                            wv_sb[:, 0, dc, :],
                            start=(dc == 0),
                            stop=(dc == DC - 1),
                        )
                    nc.vector.tensor_copy(vpA[:, kc, 0:384], ps1[:, 0:384])
                for kc in range(HKC):
                    slab = kslab_tiles[kc // 4]
                    ps2 = psa.tile([P, 512], F32, tag="psa")
                    for dc in range(DC):
                        nc.tensor.matmul(
                            ps2[:, 0:384],
                            slab[:, kc % 4, dc, :],
                            wv_sb[:, 1, dc, :],
                            start=(dc == 0),
                            stop=(dc == DC - 1),
                        )
                    nc.vector.tensor_copy(vpA[:, kc, 384:OUT], ps2[:, 0:384])
                    nc.vector.tensor_copy(vpA[:, kc, OUT : OUT + 1], ones_sc)
                    nc.vector.tensor_copy(
                        vpA[:, kc, OUT + 1 : OUT + 2], zero_sc
                    )
                    nc.gpsimd.dma_start(
                        out=vpb_in[:, kc, :], in_=vpA[:, kc, :]
                    )
                nc.gpsimd.collective_compute(
                    "AllGather",
                    mybir.AluOpType.bypass,
                    replica_groups=PAIRS,
                    ins=[vpb_in.opt()],
                    outs=[vpb_out.opt()],
                )
                # Peer-half readback: the gather output is rank-ordered,
                # so rank r's peer sits in slot 1-r. Two complementary
                # predicated DMAs keep the APs static; the skipped DMA
                # still increments the semaphore, so downstream waits
                # count identically on both ranks. Both on the sync
                # queue (single-queue writers keep the matmul wait).
                me = nc.sync.partition_id() % 2
                nc.sync.dma_start(out=vpB[:], in_=vpb_out[0], cond=me)
                nc.sync.dma_start(
                    out=vpB[:], in_=vpb_out[1], cond=(me + 1) % 2
                )

                # ---- A2: Q'^T = (x_q M)^T own half ----
                for s in range(2):
                    slab = kslab_tiles[s]
                    for oc in range(DC):
                        ps = psa.tile([P, 512], F32, tag="psa")
                        for dc in range(DC):
                            nc.tensor.matmul(
                                ps,
                                mw_sb[:, dc, oc * P : (oc + 1) * P],
                                slab[:, :, dc, :],
                                start=(dc == 0),
                                stop=(dc == DC - 1),
                            )
                        nc.vector.tensor_copy(qpt[s][:, oc, :], ps)

                # ---- scoresT: contracts over d, stationary = xh slab
                # chunks (rank-relative key order), moving = Q'^T. Runs
                # inside the psa scope (ps_sc has its own banks) so no
                # pool-close fence sits between A2 and scores.
                for bi in range(2):
                    for kc in range(KC):
                        kslab = kslab_tiles[kc // 4]
                        st = ps_sc.tile([P, 512], F32, tag="sc")
                        for dc in range(DC):
                            nc.tensor.matmul(
                                st,
                                kslab[:, kc % 4, dc, :],
                                qpt[bi][:, dc, :],
                                start=(dc == 0),
                                stop=(dc == DC - 1),
                            )
                        et = expp.tile(
                            [P, 512], FP16, tag="exp", name=f"et{bi}_{kc}"
                        )
                        nc.scalar.activation(
                            et,
                            st,
                            mybir.ActivationFunctionType.Exp,
                            scale=0.125 / M_SCALE,
                        )
                        ets[(bi, kc)] = et

            # ---- out phase: psa's banks freed above feed ps_out; the
            # pool-close fence overlaps the V-gather wait.
            with tc.tile_pool(name="ps_out", bufs=3, space="PSUM") as ps_out:
                # 8 q-chunks of 128, rotating 3 PSUM bufs. Denominator
                # run (cols 512:770) goes FIRST so the recip and the
                # 512:768 normalize overlap the 0:512 run; kc 0-7 read
                # vpA (local), kc 8-15 read vpB (gathered peer half).
                def vsrc(kc):
                    return vpA if kc < HKC else vpB

                for j in range(NQ // P):
                    bi, jj = j // 4, j % 4
                    ops = ps_out.tile(
                        [P, OUT + 2], F32, tag="out", name=f"outps{j}"
                    )
                    for kc in range(KC):
                        nc.tensor.matmul(
                            ops[:, 512 : OUT + 2],
                            ets[(bi, kc)][:, jj * P : (jj + 1) * P],
                            vsrc(kc)[:, kc % HKC, 512 : OUT + 2],
                            start=(kc == 0),
                            stop=(kc == KC - 1),
                        )
                    recip = smallp.tile([P, 1], F32, tag="recip")
                    nc.vector.reciprocal(recip, ops[:, OUT : OUT + 1])
                    ob = obp.tile([P, OUT], F32, tag="ob")
                    nc.vector.tensor_scalar_mul(
                        ob[:, 512:OUT], ops[:, 512:OUT], recip
                    )
                    for kc in range(KC):
                        nc.tensor.matmul(
                            ops[:, 0:512],
                            ets[(bi, kc)][:, jj * P : (jj + 1) * P],
                            vsrc(kc)[:, kc % HKC, 0:512],
                            start=(kc == 0),
                            stop=(kc == KC - 1),
                        )
                    nc.vector.tensor_scalar_mul(
                        ob[:, 0:512], ops[:, 0:512], recip
                    )
                    nc.sync.dma_start(
                        out=out[j * P : (j + 1) * P, :], in_=ob
                    )
    nc.finalize()
    return nc


_NC_CACHE = None


def _get_nc():
    global _NC_CACHE
    if _NC_CACHE is None:
        _NC_CACHE = build_attention_nc()
    return _NC_CACHE


def _xh_layout(a2d):
    """[D, 2048] -> [4, P, 4*DC*128], quarter-major slabs: the kc-th
    128-token quarter of a slab is a contiguous DMA prefix."""
    t = a2d.reshape(DC, P, 4, 4, P)  # dc p s q t
    t = t.transpose(2, 1, 3, 0, 4)  # s p q dc t
    return np.ascontiguousarray(t.reshape(4, P, 4 * DC * P))


def _wv_layout(a2d):
    """[D, 768] -> [P, 2*DC*384], column-half-major."""
    t = a2d.reshape(DC, P, 2, 384)  # dc p h c
    t = t.transpose(1, 2, 0, 3)  # p h dc c
    return np.ascontiguousarray(t.reshape(P, 2 * DC * 384))


def _mw_layout(a2d):
    """[D, D] -> [P, DC*D], dc-major."""
    t = a2d.reshape(DC, P, D).transpose(1, 0, 2)
    return np.ascontiguousarray(t.reshape(P, DC * D))


def make_in_maps(x, kernel):
    x = np.asarray(x, dtype=np.float32)
    w = np.asarray(kernel, dtype=np.float32)
    mw16 = (M_SCALE * (w[0] @ w[1].T)).astype(np.float16)
    mw = _mw_layout(mw16)
    wv = _wv_layout(w[2].astype(np.float16))
    in_maps = []
    for core in range(N_CORES):
        b, half = core // 2, core % 2
        xt16 = x[b].T.astype(np.float16)
        # rank-relative key order: own 1024 tokens first, then peer's
        own = xt16[:, half * NQ : (half + 1) * NQ]
        peer = xt16[:, (1 - half) * NQ : (2 - half) * NQ]
        xh = _xh_layout(np.concatenate([own, peer], axis=1))
        in_maps.append({"xh": xh, "mw": mw, "wvi": wv})
    return in_maps


def assemble_output(results):
    out = np.empty((B, N, OUT), dtype=np.float32)
    for core in range(N_CORES):
        b, half = core // 2, core % 2
        out[b, half * NQ : (half + 1) * NQ, :] = results[core]["out"]
    return out


def run_on_hw(x, kernel, trace=False):
    nc = _get_nc()
    res = run_bass_kernel_spmd(
        nc, make_in_maps(x, kernel), list(range(N_CORES)), trace=trace
    )
    return assemble_output(res.results), res


def kernel(x, kernel):
    out, _ = run_on_hw(x, kernel, trace=False)
    return out
